# revision 14
# baseline (speedup 1.0000x reference)
"""Trainium2 Bass kernel for an AttentionBlock (GroupNorm -> 1-head attention -> proj -> residual).

Problem: hidden_states (4, 512, 64, 64) fp32; GroupNorm(32 groups) then
single-head attention over S=4096 tokens with head_dim=C=512, output
projection, residual add.

Sharding: 8 cores = 4 batch elements x 2 query-halves. Each core:
 - receives the full [512, 4096] (channels x spatial) slab for its batch
   element, spatially rotated so that *its* 2048 queries are columns 0:2048
   (attention is permutation-invariant over keys, so every core can run the
   identical SPMD program);
 - computes GroupNorm + K/V for all 4096 tokens (redundant x2, cheap) and
   Q only for its half;
 - computes scores^T (keys-on-partition layout), exp, attn @ V, out-proj,
   residual -- no on-chip transposes anywhere.

Numerics: fp8(e4m3) matmul operands with DoubleRow perf mode (two 128-row
k-tiles contracted per PE pass -> ~1.5-2x TensorE throughput) and fp32 PSUM
accumulation. Weights are pre-scaled on the host (wq,wk x16; wv x8; wo x16)
to keep fp8 operands out of the subnormal range; all scales cancel through
the softmax-denominator broadcast constant (ones = 8*16 = 128). Softmax
without max-subtraction (scores ~ N(0,1)) but with a constant exp-bias of -4
to keep unnormalized sums bounded; normalization deferred past the output
projection ((P@V)@Wo / den == (P/den @ V)@Wo).
"""

from contextlib import ExitStack

import ml_dtypes
import numpy as np

import concourse.bacc as bacc
import concourse.bass as bass
import concourse.tile as tile
from concourse import mybir
from concourse.bass_utils import run_bass_kernel_spmd

F32 = mybir.dt.float32
F16 = mybir.dt.float16
F8 = mybir.dt.float8e4
F8NP = ml_dtypes.float8_e4m3
DR = mybir.MatmulPerfMode.DoubleRow

B = 4
C = 512
S = 4096  # 64*64 tokens
SH = S // 2  # tokens per core (query half)
GROUPS = 32
GSIZE = C // GROUPS  # 16 channels per group
EPS = 1e-6
CT = C // 128  # 4 channel tiles
SCALE = 1.0 / np.sqrt(np.float32(C))
EXPBIAS = -4.0  # constant max-substitute inside exp; cancels in normalization

QKSCALE = 16.0  # host pre-scale on wq/wk/bq/bk (fp8 range use)
VSCALE = 4.0  # host pre-scale on wv/bv (keeps unnormalized attn@V in fp8 range)
OSCALE = 16.0  # host pre-scale on wo
ONES_VAL = VSCALE * OSCALE  # denominator broadcast constant; cancels v/o scales
EXPSCALE = float(SCALE / (QKSCALE * QKSCALE))  # exp() input scale on raw scores

N_CORES = 8


def _build_kernel(ctx: ExitStack, tc: tile.TileContext, d):
    nc = tc.nc
    mult = mybir.AluOpType.mult
    add = mybir.AluOpType.add
    subtract = mybir.AluOpType.subtract
    Act = mybir.ActivationFunctionType

    cst = ctx.enter_context(tc.tile_pool(name="cst", bufs=1))
    xin = ctx.enter_context(tc.tile_pool(name="xin", bufs=3))
    gnp = ctx.enter_context(tc.tile_pool(name="gnp", bufs=4))
    big = ctx.enter_context(tc.tile_pool(name="big", bufs=1))
    expp = ctx.enter_context(tc.tile_pool(name="expp", bufs=4))
    smal = ctx.enter_context(tc.tile_pool(name="smal", bufs=2))
    resp = ctx.enter_context(tc.tile_pool(name="resp", bufs=2))
    finp = ctx.enter_context(tc.tile_pool(name="finp", bufs=2))

    x_d = d["x"]  # fp16 copy of the input slab: GN stats + matmul path
    # sync DMA queue order: channel tile 0 first (it heads the GroupNorm
    # pipeline), then the tiny GN constants it needs, then the other tiles.
    # Four sub-DMAs per tile so bn_stats starts on the first quarter early;
    # each tile gets its own slot so all transfers issue immediately.
    x_tiles = []
    for t in range(CT):
        x_t = xin.tile([128, S], F16, tag=f"xt{t}", name=f"xt{t}", bufs=1)
        x_tiles.append(x_t)

    def dma_x(t):
        for h in range(4):
            nc.sync.dma_start(
                out=x_tiles[t][:, h * (S // 4) : (h + 1) * (S // 4)],
                in_=x_d[t * 128 : (t + 1) * 128, h * (S // 4) : (h + 1) * (S // 4)],
            )

    dma_x(0)
    gmat_raw = cst.tile([128, 128], F32, tag="gmat_raw")
    nc.sync.dma_start(out=gmat_raw[:], in_=d["gmat"][:])
    gw_sb = cst.tile([128, CT], F32, tag="gw")
    nc.sync.dma_start(out=gw_sb[:], in_=d["gw2"][:])
    gb_sb = cst.tile([128, CT], F32, tag="gb")
    nc.sync.dma_start(out=gb_sb[:], in_=d["gb2"][:])
    for t in range(1, CT):
        dma_x(t)

    # ---- constants / weights to SBUF (gpsimd DMA queue; overlaps x).
    # Order = first-use order: K/Q/V weights gate the projections,
    # biases gate the PSUM->SBUF copies a bit later, wo3/bo much later.
    wq3 = cst.tile([128, CT, C], F8, tag="wq3")
    wk3 = cst.tile([128, CT, C], F8, tag="wk3")
    wv3 = cst.tile([128, CT, C], F8, tag="wv3")
    wo3 = cst.tile([128, CT, C], F8, tag="wo3")
    for w_sb, w_d in ((wk3, d["wkt"]), (wq3, d["wqt"]), (wv3, d["wvt"])):
        nc.gpsimd.dma_start(out=w_sb[:], in_=w_d.rearrange("(t p) o -> p t o", p=128))
    bq_sb = cst.tile([128, CT], F32, tag="bq")
    bk_sb = cst.tile([128, CT], F32, tag="bk")
    bo_sb = cst.tile([128, CT], F32, tag="bo")
    for t_sb, t_d in ((bk_sb, d["bk2"]), (bq_sb, d["bq2"]), (bo_sb, d["bo2"])):
        nc.gpsimd.dma_start(out=t_sb[:], in_=t_d[:])
    bvb_sb = cst.tile([1, C], F16, tag="bvb")
    nc.gpsimd.dma_start(out=bvb_sb[:], in_=d["bvb"][:])
    ones1_sb = cst.tile([1, 128], F16, tag="ones1")
    nc.vector.memset(ones1_sb[:], 1.0)
    nc.gpsimd.dma_start(out=wo3[:], in_=d["wot"].rearrange("(t p) o -> p t o", p=128))
    # staging copy: the first PE matmul then depends only on the DVE
    # semaphore (S3_LW allows a single wait)
    gmat_sb = cst.tile([128, 128], F32, tag="gmat")
    nc.vector.tensor_copy(out=gmat_sb[:], in_=gmat_raw[:])
    ones_sb = cst.tile([128, 128], F16, tag="ones")
    nc.vector.memset(ones_sb[:], float(ONES_VAL))
    eps_t = cst.tile([128, 1], F32, tag="epsc")
    nc.vector.memset(eps_t[:], float(EPS))
    expb_t = cst.tile([128, 1], F32, tag="expb")
    nc.vector.memset(expb_t[:], float(EXPBIAS))

    # proj-phase PSUM pool: 6 banks; scoped so its banks are released to the
    # attention pools afterwards
    proj_ctx = ExitStack()
    pjsum = proj_ctx.enter_context(tc.tile_pool(name="pjsum", bufs=6, space="PSUM"))

    # PE warmup: keep TensorE busy during the initial x DMA so HAM reaches
    # K=8/8 before real matmuls; fp16 ones matmuls, one PSUM bank, serial.
    wu = pjsum.tile([128, 128], F32, tag="wu", bufs=1)
    for _ in range(150):
        nc.tensor.matmul(wu[:], lhsT=ones_sb[:], rhs=ones_sb[:], start=True, stop=True)
    # ---- GroupNorm ----
    # Pass 1: per-partition raw [sum, sumsq] for ALL tiles -- the plain sum on
    # DVE (reduce) and the sum of squares on ACT (Square activation with
    # accum_out; its junk output lands in the xg3 slot, which the normalize
    # pass overwrites), then the group-averaging matmul. Keeping the four
    # reduces back-to-back on DVE (no per-tile chain interleaved) shortens the
    # stats pipeline by several us. The 1/(group_size*S) normalization is
    # folded into the host-provided gmat constants.
    xg3 = big.tile([128, CT, S], F8, tag="xg3")  # normalized input, [c, s]
    ps_gs = []
    for t in range(CT):
        x_t = x_tiles[t]
        mv2 = gnp.tile([128, 2], F32, tag=f"mv2_{t}", name=f"mv2_{t}", bufs=1)
        # two fp16 pairwise-fold stages (DVE 2x eligible) before the 1x final
        # reduce: ~2.7us instead of 4.4us per tile on the DVE startup chain.
        # fp16 rounding in the folds perturbs the mean by ~1e-5 -- negligible.
        sc = gnp.tile([128, 2048], F16, tag="redsc", name="redsc", bufs=2)
        nc.vector.tensor_add(out=sc[:], in0=x_t[:, 0:2048], in1=x_t[:, 2048:4096])
        nc.vector.tensor_add(out=sc[:, 0:1024], in0=sc[:, 0:1024], in1=sc[:, 1024:2048])
        nc.vector.reduce_sum(out=mv2[:, 0:1], in_=sc[:, 0:1024], axis=mybir.AxisListType.X)
        nc.scalar.activation(
            out=xg3[:, t, :], in_=x_t[:], func=Act.Square, accum_out=mv2[:, 1:2]
        )
        ps_g = pjsum.tile([128, 2], F32, tag="pj", name=f"ps_g{t}")
        nc.tensor.matmul(ps_g[:], lhsT=gmat_sb[:], rhs=mv2[:], start=True, stop=True)
        ps_gs.append(ps_g)

    # Pass 2: per-tile scale/shift chain + normalize
    for t in range(CT):
        x_t = x_tiles[t]
        ps_g = ps_gs[t]
        # gstat = [mean_g, E[x^2]_g];  var_g = E[x^2]_g - mean_g^2
        gstat = gnp.tile([128, 2], F32, tag="gstat")
        nc.vector.tensor_copy(out=gstat[:], in_=ps_g[:])
        varg = gnp.tile([128, 1], F32, tag="varg")
        nc.vector.tensor_tensor(out=varg[:], in0=gstat[:, 0:1], in1=gstat[:, 0:1], op=mult)
        nc.vector.tensor_tensor(out=varg[:], in0=gstat[:, 1:2], in1=varg[:], op=subtract)
        stdt = gnp.tile([128, 1], F32, tag="stdt")
        nc.scalar.activation(out=stdt[:], in_=varg[:], func=Act.Sqrt, bias=eps_t[:])
        rstd = gnp.tile([128, 1], F32, tag="rstd")
        nc.vector.reciprocal(out=rstd[:], in_=stdt[:])

        scl = gnp.tile([128, 1], F32, tag="scl")
        nc.vector.tensor_tensor(out=scl[:], in0=rstd[:], in1=gw_sb[:, t : t + 1], op=mult)
        sft = gnp.tile([128, 1], F32, tag="sft")
        nc.vector.tensor_tensor(out=sft[:], in0=gstat[:, 0:1], in1=scl[:], op=mult)
        nc.vector.tensor_tensor(out=sft[:], in0=gb_sb[:, t : t + 1], in1=sft[:], op=subtract)

        # normalize split 1:3 ACT:DVE -- the DVE fp16 tensor_scalar runs ~3x
        # faster per element than ACT Identity here, and ACT is already loaded
        # with the Square stats passes
        nc.scalar.activation(
            out=xg3[:, t, 0 : S // 4],
            in_=x_t[:, 0 : S // 4],
            func=Act.Identity,
            bias=sft[:],
            scale=scl[:],
        )
        nc.vector.tensor_scalar(
            out=xg3[:, t, S // 4 : S],
            in0=x_t[:, S // 4 : S],
            scalar1=scl[:],
            scalar2=sft[:],
            op0=mult,
            op1=add,
        )

    # ---- projections (fp8 DoubleRow: contract channel-tile pairs) ----
    kt3 = big.tile([128, CT, S], F8, tag="kt3")  # k^T [c, j], x QKSCALE
    for ot in range(CT):
        for jc in range(S // 512):
            ps = pjsum.tile([128, 512], F32, tag="pj")
            for tp in range(CT // 2):
                nc.tensor.matmul(
                    ps[:],
                    lhsT=wk3[:, 2 * tp : 2 * tp + 2, ot * 128 : (ot + 1) * 128],
                    rhs=xg3[:, 2 * tp : 2 * tp + 2, jc * 512 : (jc + 1) * 512],
                    start=(tp == 0),
                    stop=(tp == CT // 2 - 1),
                    perf_mode=DR,
                )
            jsl0 = slice(jc * 512, jc * 512 + 256)
            jsl1 = slice(jc * 512 + 256, (jc + 1) * 512)
            nc.scalar.activation(
                out=kt3[:, ot, jsl0],
                in_=ps[:, 0:256],
                func=Act.Identity,
                bias=bk_sb[:, ot : ot + 1],
            )
            nc.vector.tensor_scalar(
                out=kt3[:, ot, jsl1],
                in0=ps[:, 256:512],
                scalar1=bk_sb[:, ot : ot + 1],
                scalar2=None,
                op0=add,
            )

    qt3 = big.tile([128, CT, SH], F8, tag="qt3")  # q^T [c, i], x QKSCALE
    for ot in range(CT):
        for ic in range(SH // 512):
            ps = pjsum.tile([128, 512], F32, tag="pj")
            for tp in range(CT // 2):
                nc.tensor.matmul(
                    ps[:],
                    lhsT=wq3[:, 2 * tp : 2 * tp + 2, ot * 128 : (ot + 1) * 128],
                    rhs=xg3[:, 2 * tp : 2 * tp + 2, ic * 512 : (ic + 1) * 512],
                    start=(tp == 0),
                    stop=(tp == CT // 2 - 1),
                    perf_mode=DR,
                )
            isl0 = slice(ic * 512, ic * 512 + 256)
            isl1 = slice(ic * 512 + 256, (ic + 1) * 512)
            nc.scalar.activation(
                out=qt3[:, ot, isl0],
                in_=ps[:, 0:256],
                func=Act.Identity,
                bias=bq_sb[:, ot : ot + 1],
            )
            nc.vector.tensor_scalar(
                out=qt3[:, ot, isl1],
                in0=ps[:, 256:512],
                scalar1=bq_sb[:, ot : ot + 1],
                scalar2=None,
                op0=add,
            )

    v3 = big.tile([128, S // 128, C], F8, tag="v3")  # v natural [j, o], x VSCALE
    for jb in range(S // 128):
        ps = pjsum.tile([128, 512], F32, tag="pj")
        # bias init: broadcast bv over the 128 token partitions via a K=1 matmul
        nc.tensor.matmul(ps[:], lhsT=ones1_sb[:], rhs=bvb_sb[:], start=True, stop=False)
        for tp in range(CT // 2):
            nc.tensor.matmul(
                ps[:],
                lhsT=xg3[:, 2 * tp : 2 * tp + 2, jb * 128 : (jb + 1) * 128],
                rhs=wv3[:, 2 * tp : 2 * tp + 2, :],
                start=False,
                stop=(tp == CT // 2 - 1),
                perf_mode=DR,
            )
        nc.vector.tensor_copy(out=v3[:, jb, 0:256], in_=ps[:, 0:256])
        nc.scalar.activation(out=v3[:, jb, 256:512], in_=ps[:, 256:512], func=Act.Copy)

    # release the 6 proj banks, then open the attention PSUM pools:
    # ps pairs (2 banks x 2 bufs) + av0..3 (1 each) = 8 banks. The finisher's
    # denominator/out-proj PSUM shares the "ps" rotation.
    proj_ctx.close()
    ppsum = ctx.enter_context(tc.tile_pool(name="ppsum", bufs=2, space="PSUM"))
    apsum = ctx.enter_context(tc.tile_pool(name="apsum", bufs=1, space="PSUM"))

    # ---- attention + output projection, per 512-query chunk ----
    # The per-chunk epilogue (denominator, attn-out copies, output projection,
    # residual) is deferred into the next chunk's j-loop so its PE work and
    # PSUM->SBUF copies overlap the next chunk's score matmuls.
    NJP = S // 256  # 16 key-block pairs

    def make_finisher(ic, av, sums_f):
        isl = slice(ic * 512, (ic + 1) * 512)
        state = {}

        def finish_a():
            # denominator broadcast to all partitions via ones-matmul
            ps_den = ppsum.tile([128, 2, 512], F32, tag="ps", name="ps_den")
            nc.tensor.matmul(
                ps_den[:, 0, :], lhsT=ones_sb[:], rhs=sums_f[:], start=True, stop=True
            )
            # PSUM->SBUF attn-out copies gate the next chunk's attnV (av bank
            # reuse): split each copy half DVE / half ACT to halve the stall.
            a4 = smal.tile([128, CT, 512], F8, tag="a4", name="a4")
            for ot in range(CT):
                nc.vector.tensor_copy(out=a4[:, ot, 0:256], in_=av[ot][:, 0:256])
                nc.scalar.activation(
                    out=a4[:, ot, 256:512], in_=av[ot][:, 256:512], func=Act.Copy
                )
            state["ps_den"] = ps_den
            state["a4"] = a4

        def finish_b():
            ps_den, a4 = state["ps_den"], state["a4"]
            recip = smal.tile([128, 512], F32, tag="recip", name="recip")
            nc.vector.reciprocal(out=recip[:], in_=ps_den[:, 0, :])

            ps_o = None
            for ot2 in range(CT):
                osl = slice(ot2 * 128, (ot2 + 1) * 128)
                h = ot2 % 2
                if h == 0:
                    ps_o = ppsum.tile([128, 2, 512], F32, tag="ps", name="ps_o")
                for tp in range(CT // 2):
                    nc.tensor.matmul(
                        ps_o[:, h, :],
                        lhsT=wo3[:, 2 * tp : 2 * tp + 2, osl],
                        rhs=a4[:, 2 * tp : 2 * tp + 2, :],
                        start=(tp == 0),
                        stop=(tp == CT // 2 - 1),
                        perf_mode=DR,
                    )
                res_t = resp.tile([128, 512], F32, tag="res", name="res_t")
                nc.sync.dma_start(out=res_t[:], in_=d["xr"][osl, isl])
                f1 = finp.tile([128, 512], F32, tag="f1", name="f1")
                nc.vector.tensor_tensor(out=f1[:], in0=ps_o[:, h, :], in1=recip[:], op=mult)
                nc.vector.scalar_tensor_tensor(
                    out=f1[:],
                    in0=f1[:],
                    scalar=bo_sb[:, ot2 : ot2 + 1],
                    in1=res_t[:],
                    op0=add,
                    op1=add,
                )
                nc.sync.dma_start(out=d["out"][osl, isl], in_=f1[:])

        return finish_a, finish_b

    finish_prev = None
    for ic in range(SH // 512):
        isl = slice(ic * 512, (ic + 1) * 512)
        av = [
            apsum.tile([128, 512], F32, tag=f"av{ot}", name=f"av{ot}")
            for ot in range(CT)
        ]
        # denominator tree: leaf m sums key-block pairs 2m,2m+1; folds are
        # in-place (L[2i] += L[2i+1] etc). Same-dtype adds keep DVE in 2x mode.
        lv = [
            smal.tile([128, 2, 512], F16, tag=f"lv{m}", name=f"lv{m}", bufs=1)
            for m in range(NJP // 2)
        ]
        sums_f = smal.tile([128, 512], F16, tag="sums_f", name="sums_f")

        def scores_exp(jp):
            # scores^T for key blocks 2jp, 2jp+1 into a 2-bank PSUM pair tile;
            # one 1024-wide exp ACTIVATE writes the fp8 pair layout attnV needs.
            ps_s = ppsum.tile([128, 2, 512], F32, tag="ps", name="ps_s")
            for h in range(2):
                jb = 2 * jp + h
                for tp in range(CT // 2):
                    nc.tensor.matmul(
                        ps_s[:, h, :],
                        lhsT=kt3[:, 2 * tp : 2 * tp + 2, jb * 128 : (jb + 1) * 128],
                        rhs=qt3[:, 2 * tp : 2 * tp + 2, isl],
                        start=(tp == 0),
                        stop=(tp == CT // 2 - 1),
                        perf_mode=DR,
                    )
            e_t = expp.tile([128, 2, 512], F8, tag="exp", name="e_t")
            nc.scalar.activation(
                out=e_t[:], in_=ps_s[:], func=Act.Exp, bias=expb_t[:], scale=EXPSCALE
            )
            return e_t

        e_hold = [None]

        def attnv_sums(jp, e_t):
            for ot in range(CT):
                nc.tensor.matmul(
                    av[ot][:],
                    lhsT=v3[:, 2 * jp : 2 * jp + 2, ot * 128 : (ot + 1) * 128],
                    rhs=e_t[:],
                    start=(jp == 0),
                    stop=(jp == NJP - 1),
                    perf_mode=DR,
                )
            if jp % 2 == 0:
                e_hold[0] = e_t
            else:
                m = jp // 2
                nc.vector.tensor_add(out=lv[m][:], in0=e_hold[0][:], in1=e_t[:])
                if m % 2 == 1:
                    nc.vector.tensor_add(out=lv[m - 1][:], in0=lv[m - 1][:], in1=lv[m][:])
                if m == 3:
                    nc.vector.tensor_add(out=lv[0][:], in0=lv[0][:], in1=lv[2][:])
                if m == 7:
                    nc.vector.tensor_add(out=lv[4][:], in0=lv[4][:], in1=lv[6][:])
                    nc.vector.tensor_add(out=lv[0][:], in0=lv[0][:], in1=lv[4][:])
                    nc.vector.tensor_add(
                        out=sums_f[:], in0=lv[0][:, 0, :], in1=lv[0][:, 1, :]
                    )

        # Boundary order: two score pair-groups first, then the previous
        # chunk's epilogue part A (denominator + the av-bank-releasing
        # copies), then this chunk's first attnV. Epilogue part B (reciprocal
        # + out-proj + residual fuse) is emitted after several j-iterations so
        # its DVE work queues behind this chunk's sums ops (which release
        # exp-pool slots the score pipeline needs).
        e0 = scores_exp(0)
        e1 = scores_exp(1)
        if finish_prev is not None:
            finish_prev[0]()
        attnv_sums(0, e0)
        attnv_sums(1, e1)
        for jp in range(2, NJP):
            e_t = scores_exp(jp)
            attnv_sums(jp, e_t)
            if jp == 3 and finish_prev is not None:
                finish_prev[1]()
                finish_prev = None
        finish_prev = make_finisher(ic, av, sums_f)
    finish_prev[0]()
    finish_prev[1]()


_CACHE = {}


def _get_program():
    if "nc" in _CACHE:
        return _CACHE["nc"]
    nc = bacc.Bacc("TRN2", target_bir_lowering=False, debug=False, num_devices=N_CORES)
    d = {}
    d["x"] = nc.dram_tensor("x", [C, S], F16, kind="ExternalInput").ap()
    d["xr"] = nc.dram_tensor("xr", [C, SH], F32, kind="ExternalInput").ap()
    for name in ("wqt", "wkt", "wvt", "wot"):
        d[name] = nc.dram_tensor(name, [C, C], F8, kind="ExternalInput").ap()
    for name in ("bq2", "bk2", "bo2", "gw2", "gb2"):
        d[name] = nc.dram_tensor(name, [128, CT], F32, kind="ExternalInput").ap()
    d["bvb"] = nc.dram_tensor("bvb", [1, C], F16, kind="ExternalInput").ap()
    d["gmat"] = nc.dram_tensor("gmat", [128, 128], F32, kind="ExternalInput").ap()
    d["out"] = nc.dram_tensor("out", [C, SH], F32, kind="ExternalOutput").ap()

    with tile.TileContext(nc) as tc:
        with ExitStack() as ctx:
            _build_kernel(ctx, tc, d)
    nc.compile()
    _CACHE["nc"] = nc
    return nc


def make_in_maps(**inputs):
    """Per-core input dicts (numpy). Core c handles batch c//2, query-half c%2."""
    f32 = np.float32
    hs = np.asarray(inputs["hidden_states"], f32).reshape(B, C, S)
    common = {}
    for wname, key, ws in (
        ("wqt", "wq", QKSCALE),
        ("wkt", "wk", QKSCALE),
        ("wvt", "wv", VSCALE),
        ("wot", "wo", OSCALE),
    ):
        w = np.asarray(inputs[key], f32) * ws
        common[wname] = np.ascontiguousarray(w.T).astype(F8NP)
    for bname, key, bs in (
        ("bq2", "bq", QKSCALE),
        ("bk2", "bk", QKSCALE),
        ("bo2", "bo", 1.0),
    ):
        b = np.asarray(inputs[key], f32) * bs
        common[bname] = np.ascontiguousarray(b.reshape(CT, 128).T)
    common["gw2"] = np.ascontiguousarray(
        np.asarray(inputs["gn_weight"], f32).reshape(CT, 128).T
    )
    common["gb2"] = np.ascontiguousarray(
        np.asarray(inputs["gn_bias"], f32).reshape(CT, 128).T
    )
    common["bvb"] = np.ascontiguousarray(
        (np.asarray(inputs["bv"], f32) * VSCALE).reshape(1, C).astype(np.float16)
    )
    gmat = np.zeros((128, 128), f32)
    for g in range(128 // GSIZE):
        # averages raw per-partition [sum, sumsq] into per-group [mean, E[x^2]]
        gmat[g * GSIZE : (g + 1) * GSIZE, g * GSIZE : (g + 1) * GSIZE] = 1.0 / (
            GSIZE * S
        )
    common["gmat"] = gmat

    in_maps = []
    for core in range(N_CORES):
        b_idx, half = divmod(core, 2)
        xb = hs[b_idx]
        if half:
            xp = np.concatenate([xb[:, SH:], xb[:, :SH]], axis=1)
        else:
            xp = xb
        m = dict(common)
        m["x"] = np.ascontiguousarray(xp.astype(np.float16))
        m["xr"] = np.ascontiguousarray(xp[:, :SH])
        in_maps.append(m)
    return in_maps


def assemble_output(results):
    out = np.empty((B, C, S), np.float32)
    for core in range(N_CORES):
        b_idx, half = divmod(core, 2)
        out[b_idx][:, half * SH : (half + 1) * SH] = results[core]["out"]
    return out.reshape(B, C, 64, 64)


def run(trace=False, **inputs):
    nc = _get_program()
    in_maps = make_in_maps(**inputs)
    res = run_bass_kernel_spmd(nc, in_maps, core_ids=list(range(N_CORES)), trace=trace)
    return assemble_output(res.results), res


def kernel(**inputs):
    out, _ = run(**inputs)
    return out


# revision 19
# speedup vs baseline: 1.2926x; 1.2926x over previous
"""Trainium2 Bass kernel for an AttentionBlock (GroupNorm -> 1-head attention -> proj -> residual).

Problem: hidden_states (4, 512, 64, 64) fp32; GroupNorm(32 groups) then
single-head attention over S=4096 tokens with head_dim=C=512, output
projection, residual add.

Sharding: 8 cores = 4 batch elements x 2 query-halves. Each core:
 - receives the full [512, 4096] (channels x spatial) slab for its batch
   element, spatially rotated so that *its* 2048 queries are columns 0:2048
   (attention is permutation-invariant over keys, so every core can run the
   identical SPMD program);
 - computes GroupNorm + K/V for all 4096 tokens (redundant x2, cheap) and
   Q only for its half;
 - computes scores^T (keys-on-partition layout), exp, attn @ V, out-proj,
   residual -- no on-chip transposes anywhere.

Numerics: fp8(e4m3) matmul operands with DoubleRow perf mode (two 128-row
k-tiles contracted per PE pass -> ~1.5-2x TensorE throughput) and fp32 PSUM
accumulation. Weights are pre-scaled on the host (wq,wk x16; wv x8; wo x16)
to keep fp8 operands out of the subnormal range; all scales cancel through
the softmax-denominator broadcast constant (ones = 8*16 = 128). Softmax
without max-subtraction (scores ~ N(0,1)) but with a constant exp-bias of -4
to keep unnormalized sums bounded; normalization deferred past the output
projection ((P@V)@Wo / den == (P/den @ V)@Wo).
"""

from contextlib import ExitStack

import ml_dtypes
import numpy as np

import concourse.bacc as bacc
import concourse.bass as bass
import concourse.tile as tile
from concourse import mybir
from concourse.bass_utils import run_bass_kernel_spmd

F32 = mybir.dt.float32
F16 = mybir.dt.float16
F8 = mybir.dt.float8e4
F8NP = ml_dtypes.float8_e4m3
DR = mybir.MatmulPerfMode.DoubleRow

B = 4
C = 512
S = 4096  # 64*64 tokens
SH = S // 2  # tokens per core (query half)
GROUPS = 32
GSIZE = C // GROUPS  # 16 channels per group
EPS = 1e-6
CT = C // 128  # 4 channel tiles
SCALE = 1.0 / np.sqrt(np.float32(C))
EXPBIAS = -4.0  # constant max-substitute inside exp; cancels in normalization

QKSCALE = 16.0  # host pre-scale on wq/wk/bq/bk (fp8 range use)
VSCALE = 4.0  # host pre-scale on wv/bv (keeps unnormalized attn@V in fp8 range)
OSCALE = 16.0  # host pre-scale on wo
ONES_VAL = VSCALE * OSCALE  # denominator broadcast constant; cancels v/o scales
EXPSCALE = float(SCALE / (QKSCALE * QKSCALE))  # exp() input scale on raw scores

N_CORES = 8


def _build_kernel(ctx: ExitStack, tc: tile.TileContext, d):
    nc = tc.nc
    mult = mybir.AluOpType.mult
    add = mybir.AluOpType.add
    subtract = mybir.AluOpType.subtract
    Act = mybir.ActivationFunctionType

    cst = ctx.enter_context(tc.tile_pool(name="cst", bufs=1))
    xin = ctx.enter_context(tc.tile_pool(name="xin", bufs=3))
    gnp = ctx.enter_context(tc.tile_pool(name="gnp", bufs=4))
    big = ctx.enter_context(tc.tile_pool(name="big", bufs=1))
    expp = ctx.enter_context(tc.tile_pool(name="expp", bufs=4))
    smal = ctx.enter_context(tc.tile_pool(name="smal", bufs=2))
    resp = ctx.enter_context(tc.tile_pool(name="resp", bufs=2))
    finp = ctx.enter_context(tc.tile_pool(name="finp", bufs=2))

    x_d = d["x"]  # fp16 copy of the input slab: GN stats + matmul path
    # sync DMA queue order: channel tile 0 first (it heads the GroupNorm
    # pipeline), then the tiny GN constants it needs, then the other tiles.
    # Four sub-DMAs per tile so bn_stats starts on the first quarter early;
    # each tile gets its own slot so all transfers issue immediately.
    x_tiles = []
    for t in range(CT):
        x_t = xin.tile([128, S], F16, tag=f"xt{t}", name=f"xt{t}", bufs=1)
        x_tiles.append(x_t)

    def dma_x(t):
        for h in range(4):
            nc.sync.dma_start(
                out=x_tiles[t][:, h * (S // 4) : (h + 1) * (S // 4)],
                in_=x_d[t * 128 : (t + 1) * 128, h * (S // 4) : (h + 1) * (S // 4)],
            )

    dma_x(0)
    gmat_raw = cst.tile([128, 128], F32, tag="gmat_raw")
    nc.sync.dma_start(out=gmat_raw[:], in_=d["gmat"][:])
    gw_sb = cst.tile([128, CT], F32, tag="gw")
    nc.sync.dma_start(out=gw_sb[:], in_=d["gw2"][:])
    gb_sb = cst.tile([128, CT], F32, tag="gb")
    nc.sync.dma_start(out=gb_sb[:], in_=d["gb2"][:])
    for t in range(1, CT):
        dma_x(t)

    # ---- constants / weights to SBUF (gpsimd DMA queue; overlaps x).
    # Order = first-use order: K/Q/V weights gate the projections,
    # biases gate the PSUM->SBUF copies a bit later, wo3/bo much later.
    wq3 = cst.tile([128, CT, C], F8, tag="wq3")
    wk3 = cst.tile([128, CT, C], F8, tag="wk3")
    wv3 = cst.tile([128, CT, C], F8, tag="wv3")
    wo3 = cst.tile([128, CT, C], F8, tag="wo3")
    for w_sb, w_d in ((wk3, d["wkt"]), (wq3, d["wqt"]), (wv3, d["wvt"])):
        nc.gpsimd.dma_start(out=w_sb[:], in_=w_d.rearrange("(t p) o -> p t o", p=128))
    bq_sb = cst.tile([128, CT], F32, tag="bq")
    bk_sb = cst.tile([128, CT], F32, tag="bk")
    bo_sb = cst.tile([128, CT], F32, tag="bo")
    for t_sb, t_d in ((bk_sb, d["bk2"]), (bq_sb, d["bq2"]), (bo_sb, d["bo2"])):
        nc.gpsimd.dma_start(out=t_sb[:], in_=t_d[:])
    nc.gpsimd.dma_start(out=wo3[:], in_=d["wot"].rearrange("(t p) o -> p t o", p=128))
    # staging copy: the first PE matmul then depends only on the DVE
    # semaphore (S3_LW allows a single wait)
    gmat_sb = cst.tile([128, 128], F32, tag="gmat")
    nc.vector.tensor_copy(out=gmat_sb[:], in_=gmat_raw[:])
    ones8 = cst.tile([128, 2, 128], F8, tag="ones8")
    nc.vector.memset(ones8[:], float(ONES_VAL))
    eps_t = cst.tile([128, 1], F32, tag="epsc")
    nc.vector.memset(eps_t[:], float(EPS))
    expb_t = cst.tile([128, 1], F32, tag="expb")
    nc.vector.memset(expb_t[:], float(EXPBIAS))

    # proj-phase PSUM pool: 6 banks; scoped so its banks are released to the
    # attention pools afterwards
    proj_ctx = ExitStack()
    pjsum = proj_ctx.enter_context(tc.tile_pool(name="pjsum", bufs=6, space="PSUM"))

    # PE warmup: keep TensorE busy during the initial x DMA so HAM reaches
    # K=8/8 before real matmuls; fp16 ones matmuls, one PSUM bank, serial.
    wu = pjsum.tile([128, 128], F32, tag="wu", bufs=1)
    for _ in range(150):
        nc.tensor.matmul(
            wu[:], lhsT=ones8[:, 0, :], rhs=ones8[:, 0, :], start=True, stop=True
        )
    # ---- GroupNorm ----
    # Pass 1: per-partition raw [sum, sumsq] for ALL tiles -- the plain sum on
    # DVE (reduce) and the sum of squares on ACT (Square activation with
    # accum_out; its junk output lands in the xg3 slot, which the normalize
    # pass overwrites), then the group-averaging matmul. Keeping the four
    # reduces back-to-back on DVE (no per-tile chain interleaved) shortens the
    # stats pipeline by several us. The 1/(group_size*S) normalization is
    # folded into the host-provided gmat constants.
    xg3 = big.tile([128, CT, S], F8, tag="xg3")  # normalized input, [c, s]
    ps_gs = []
    for t in range(CT):
        x_t = x_tiles[t]
        mv2 = gnp.tile([128, 2], F32, tag=f"mv2_{t}", name=f"mv2_{t}", bufs=1)
        # two fp16 pairwise-fold stages (DVE 2x eligible) before the 1x final
        # reduce: ~2.7us instead of 4.4us per tile on the DVE startup chain.
        # fp16 rounding in the folds perturbs the mean by ~1e-5 -- negligible.
        sc = gnp.tile([128, 2048], F16, tag="redsc", name="redsc", bufs=2)
        nc.vector.tensor_add(out=sc[:], in0=x_t[:, 0:2048], in1=x_t[:, 2048:4096])
        nc.vector.tensor_add(out=sc[:, 0:1024], in0=sc[:, 0:1024], in1=sc[:, 1024:2048])
        nc.vector.reduce_sum(out=mv2[:, 0:1], in_=sc[:, 0:1024], axis=mybir.AxisListType.X)
        nc.scalar.activation(
            out=xg3[:, t, :], in_=x_t[:], func=Act.Square, accum_out=mv2[:, 1:2]
        )
        ps_g = pjsum.tile([128, 2], F32, tag="pj", name=f"ps_g{t}")
        nc.tensor.matmul(ps_g[:], lhsT=gmat_sb[:], rhs=mv2[:], start=True, stop=True)
        ps_gs.append(ps_g)

    # Pass 2: per-tile scale/shift chain + normalize
    for t in range(CT):
        x_t = x_tiles[t]
        ps_g = ps_gs[t]
        # gstat = [mean_g, E[x^2]_g];  var_g = E[x^2]_g - mean_g^2
        gstat = gnp.tile([128, 2], F32, tag="gstat")
        nc.vector.tensor_copy(out=gstat[:], in_=ps_g[:])
        varg = gnp.tile([128, 1], F32, tag="varg")
        nc.vector.tensor_tensor(out=varg[:], in0=gstat[:, 0:1], in1=gstat[:, 0:1], op=mult)
        nc.vector.tensor_tensor(out=varg[:], in0=gstat[:, 1:2], in1=varg[:], op=subtract)
        stdt = gnp.tile([128, 1], F32, tag="stdt")
        nc.scalar.activation(out=stdt[:], in_=varg[:], func=Act.Sqrt, bias=eps_t[:])
        rstd = gnp.tile([128, 1], F32, tag="rstd")
        nc.vector.reciprocal(out=rstd[:], in_=stdt[:])

        scl = gnp.tile([128, 1], F32, tag="scl")
        nc.vector.tensor_tensor(out=scl[:], in0=rstd[:], in1=gw_sb[:, t : t + 1], op=mult)
        sft = gnp.tile([128, 1], F32, tag="sft")
        nc.vector.tensor_tensor(out=sft[:], in0=gstat[:, 0:1], in1=scl[:], op=mult)
        nc.vector.tensor_tensor(out=sft[:], in0=gb_sb[:, t : t + 1], in1=sft[:], op=subtract)

        # normalize split 1:3 ACT:DVE -- the DVE fp16 tensor_scalar runs ~3x
        # faster per element than ACT Identity here, and ACT is already loaded
        # with the Square stats passes
        nc.scalar.activation(
            out=xg3[:, t, 0 : S // 4],
            in_=x_t[:, 0 : S // 4],
            func=Act.Identity,
            bias=sft[:],
            scale=scl[:],
        )
        nc.vector.tensor_scalar(
            out=xg3[:, t, S // 4 : S],
            in0=x_t[:, S // 4 : S],
            scalar1=scl[:],
            scalar2=sft[:],
            op0=mult,
            op1=add,
        )

    # ---- projections (fp8 DoubleRow: contract channel-tile pairs) ----
    kt3 = big.tile([128, CT, S], F8, tag="kt3")  # k^T [c, j], x QKSCALE
    for ot in range(CT):
        for jc in range(S // 512):
            ps = pjsum.tile([128, 512], F32, tag="pj")
            for tp in range(CT // 2):
                nc.tensor.matmul(
                    ps[:],
                    lhsT=wk3[:, 2 * tp : 2 * tp + 2, ot * 128 : (ot + 1) * 128],
                    rhs=xg3[:, 2 * tp : 2 * tp + 2, jc * 512 : (jc + 1) * 512],
                    start=(tp == 0),
                    stop=(tp == CT // 2 - 1),
                    perf_mode=DR,
                )
            jsl0 = slice(jc * 512, jc * 512 + 256)
            jsl1 = slice(jc * 512 + 256, (jc + 1) * 512)
            nc.scalar.activation(
                out=kt3[:, ot, jsl0],
                in_=ps[:, 0:256],
                func=Act.Identity,
                bias=bk_sb[:, ot : ot + 1],
            )
            nc.vector.tensor_scalar(
                out=kt3[:, ot, jsl1],
                in0=ps[:, 256:512],
                scalar1=bk_sb[:, ot : ot + 1],
                scalar2=None,
                op0=add,
            )

    qt3 = big.tile([128, CT, SH], F8, tag="qt3")  # q^T [c, i], x QKSCALE
    for ot in range(CT):
        for ic in range(SH // 512):
            ps = pjsum.tile([128, 512], F32, tag="pj")
            for tp in range(CT // 2):
                nc.tensor.matmul(
                    ps[:],
                    lhsT=wq3[:, 2 * tp : 2 * tp + 2, ot * 128 : (ot + 1) * 128],
                    rhs=xg3[:, 2 * tp : 2 * tp + 2, ic * 512 : (ic + 1) * 512],
                    start=(tp == 0),
                    stop=(tp == CT // 2 - 1),
                    perf_mode=DR,
                )
            isl0 = slice(ic * 512, ic * 512 + 256)
            isl1 = slice(ic * 512 + 256, (ic + 1) * 512)
            nc.scalar.activation(
                out=qt3[:, ot, isl0],
                in_=ps[:, 0:256],
                func=Act.Identity,
                bias=bq_sb[:, ot : ot + 1],
            )
            nc.vector.tensor_scalar(
                out=qt3[:, ot, isl1],
                in0=ps[:, 256:512],
                scalar1=bq_sb[:, ot : ot + 1],
                scalar2=None,
                op0=add,
            )

    v3 = big.tile([128, S // 128, C], F8, tag="v3")  # v natural [j, o], x VSCALE
    for jb in range(S // 128):
        ps = pjsum.tile([128, 512], F32, tag="pj")
        for tp in range(CT // 2):
            nc.tensor.matmul(
                ps[:],
                lhsT=xg3[:, 2 * tp : 2 * tp + 2, jb * 128 : (jb + 1) * 128],
                rhs=wv3[:, 2 * tp : 2 * tp + 2, :],
                start=(tp == 0),
                stop=(tp == CT // 2 - 1),
                perf_mode=DR,
            )
        nc.vector.tensor_copy(out=v3[:, jb, 0:256], in_=ps[:, 0:256])
        nc.scalar.activation(out=v3[:, jb, 256:512], in_=ps[:, 256:512], func=Act.Copy)

    # release the 6 proj banks, then open the attention PSUM pools:
    # ps pairs (2 banks x 2 bufs) + av0..3 (1 each) = 8 banks. The finisher's
    # denominator/out-proj PSUM shares the "ps" rotation.
    proj_ctx.close()
    ppsum = ctx.enter_context(tc.tile_pool(name="ppsum", bufs=3, space="PSUM"))
    dpsum = ctx.enter_context(tc.tile_pool(name="dpsum", bufs=1, space="PSUM"))
    apsum = ctx.enter_context(tc.tile_pool(name="apsum", bufs=1, space="PSUM"))

    # ---- attention + output projection, per 512-query chunk ----
    # The per-chunk epilogue (denominator, attn-out copies, output projection,
    # residual) is deferred into the next chunk's j-loop so its PE work and
    # PSUM->SBUF copies overlap the next chunk's score matmuls.
    NJP = S // 256  # 16 key-block pairs

    def make_finisher(ic, av, den_ps):
        isl = slice(ic * 512, (ic + 1) * 512)
        state = {}

        def finish_a():
            # reciprocal straight off the PE-accumulated denominator bank
            recip = smal.tile([128, 512], F32, tag="recip", name="recip")
            nc.vector.reciprocal(out=recip[:], in_=den_ps[:])
            # PSUM->SBUF attn-out copies gate the next chunk's attnV (av bank
            # reuse): split each copy half DVE / half ACT to halve the stall.
            a4 = smal.tile([128, CT, 512], F8, tag="a4", name="a4")
            for ot in range(CT):
                nc.vector.tensor_copy(out=a4[:, ot, 0:256], in_=av[ot][:, 0:256])
                nc.scalar.activation(
                    out=a4[:, ot, 256:512], in_=av[ot][:, 256:512], func=Act.Copy
                )
            state["recip"] = recip
            state["a4"] = a4

        def finish_b():
            recip, a4 = state["recip"], state["a4"]
            for ot2 in range(CT):
                osl = slice(ot2 * 128, (ot2 + 1) * 128)
                ps_o = ppsum.tile([128, 512], F32, tag="ps", name="ps_o")
                for tp in range(CT // 2):
                    nc.tensor.matmul(
                        ps_o[:],
                        lhsT=wo3[:, 2 * tp : 2 * tp + 2, osl],
                        rhs=a4[:, 2 * tp : 2 * tp + 2, :],
                        start=(tp == 0),
                        stop=(tp == CT // 2 - 1),
                        perf_mode=DR,
                    )
                res_t = resp.tile([128, 512], F32, tag="res", name="res_t")
                nc.sync.dma_start(out=res_t[:], in_=d["xr"][osl, isl])
                f1 = finp.tile([128, 512], F32, tag="f1", name="f1")
                nc.vector.tensor_tensor(out=f1[:], in0=ps_o[:], in1=recip[:], op=mult)
                nc.vector.scalar_tensor_tensor(
                    out=f1[:],
                    in0=f1[:],
                    scalar=bo_sb[:, ot2 : ot2 + 1],
                    in1=res_t[:],
                    op0=add,
                    op1=add,
                )
                nc.sync.dma_start(out=d["out"][osl, isl], in_=f1[:])

        return finish_a, finish_b

    finish_prev = None
    for ic in range(SH // 512):
        isl = slice(ic * 512, (ic + 1) * 512)
        av = [
            apsum.tile([128, 512], F32, tag=f"av{ot}", name=f"av{ot}")
            for ot in range(CT)
        ]
        den_ps = dpsum.tile([128, 512], F32, tag="den", name="den_ps")

        def scores_exp(jp):
            # scores^T for key blocks 2jp, 2jp+1: two single-bank PSUM tiles,
            # two 512-wide exps into the fp8 pair-layout tile attnV reads.
            e_t = expp.tile([128, 2, 512], F8, tag="exp", name="e_t")
            for h in range(2):
                jb = 2 * jp + h
                ps_s = ppsum.tile([128, 512], F32, tag="ps", name="ps_s")
                for tp in range(CT // 2):
                    nc.tensor.matmul(
                        ps_s[:],
                        lhsT=kt3[:, 2 * tp : 2 * tp + 2, jb * 128 : (jb + 1) * 128],
                        rhs=qt3[:, 2 * tp : 2 * tp + 2, isl],
                        start=(tp == 0),
                        stop=(tp == CT // 2 - 1),
                        perf_mode=DR,
                    )
                nc.scalar.activation(
                    out=e_t[:, h, :],
                    in_=ps_s[:],
                    func=Act.Exp,
                    bias=expb_t[:],
                    scale=EXPSCALE,
                )
            return e_t

        es = {}

        def attnv_sums(jp, e_t):
            for ot in range(CT):
                nc.tensor.matmul(
                    av[ot][:],
                    lhsT=v3[:, 2 * jp : 2 * jp + 2, ot * 128 : (ot + 1) * 128],
                    rhs=e_t[:],
                    start=(jp == 0),
                    stop=(jp == NJP - 1),
                    perf_mode=DR,
                )
            # denominator accumulation on PE, one pair behind (gives the
            # previous chunk's reciprocal time to read the den bank)
            if jp > 0:
                nc.tensor.matmul(
                    den_ps[:],
                    lhsT=ones8[:],
                    rhs=es[jp - 1][:],
                    start=(jp == 1),
                    stop=False,
                    perf_mode=DR,
                )

        # Depth-3 score pipeline across the chunk boundary: three pair-groups
        # of scores queue on PE before the first attnV (which must wait for
        # the previous chunk's a4 copies to release the av banks).
        es[0] = scores_exp(0)
        es[1] = scores_exp(1)
        if finish_prev is not None:
            finish_prev[0]()
        es[2] = scores_exp(2)
        for jp in range(NJP):
            attnv_sums(jp, es[jp])
            if jp + 3 < NJP:
                es[jp + 3] = scores_exp(jp + 3)
            if jp == 3 and finish_prev is not None:
                finish_prev[1]()
                finish_prev = None
        nc.tensor.matmul(
            den_ps[:],
            lhsT=ones8[:],
            rhs=es[NJP - 1][:],
            start=False,
            stop=True,
            perf_mode=DR,
        )
        es.clear()
        finish_prev = make_finisher(ic, av, den_ps)
    finish_prev[0]()
    finish_prev[1]()


_CACHE = {}


def _get_program():
    if "nc" in _CACHE:
        return _CACHE["nc"]
    nc = bacc.Bacc("TRN2", target_bir_lowering=False, debug=False, num_devices=N_CORES)
    d = {}
    d["x"] = nc.dram_tensor("x", [C, S], F16, kind="ExternalInput").ap()
    d["xr"] = nc.dram_tensor("xr", [C, SH], F32, kind="ExternalInput").ap()
    for name in ("wqt", "wkt", "wvt", "wot"):
        d[name] = nc.dram_tensor(name, [C, C], F8, kind="ExternalInput").ap()
    for name in ("bq2", "bk2", "bo2", "gw2", "gb2"):
        d[name] = nc.dram_tensor(name, [128, CT], F32, kind="ExternalInput").ap()
    d["gmat"] = nc.dram_tensor("gmat", [128, 128], F32, kind="ExternalInput").ap()
    d["out"] = nc.dram_tensor("out", [C, SH], F32, kind="ExternalOutput").ap()

    with tile.TileContext(nc) as tc:
        with ExitStack() as ctx:
            _build_kernel(ctx, tc, d)
    nc.compile()
    _CACHE["nc"] = nc
    return nc


def make_in_maps(**inputs):
    """Per-core input dicts (numpy). Core c handles batch c//2, query-half c%2."""
    f32 = np.float32
    hs = np.asarray(inputs["hidden_states"], f32).reshape(B, C, S)
    common = {}
    for wname, key, ws in (
        ("wqt", "wq", QKSCALE),
        ("wkt", "wk", QKSCALE),
        ("wvt", "wv", VSCALE),
        ("wot", "wo", OSCALE),
    ):
        w = np.asarray(inputs[key], f32) * ws
        common[wname] = np.ascontiguousarray(w.T).astype(F8NP)
    # bv folds into the output-projection bias: out += (P @ 1*bv) @ Wo.T
    bo_eff = np.asarray(inputs["bo"], f32) + np.asarray(inputs["wo"], f32) @ np.asarray(
        inputs["bv"], f32
    )
    for bname, bvec, bs in (
        ("bq2", np.asarray(inputs["bq"], f32), QKSCALE),
        ("bk2", np.asarray(inputs["bk"], f32), QKSCALE),
        ("bo2", bo_eff, 1.0),
    ):
        b = bvec * bs
        common[bname] = np.ascontiguousarray(b.reshape(CT, 128).T)
    common["gw2"] = np.ascontiguousarray(
        np.asarray(inputs["gn_weight"], f32).reshape(CT, 128).T
    )
    common["gb2"] = np.ascontiguousarray(
        np.asarray(inputs["gn_bias"], f32).reshape(CT, 128).T
    )
    gmat = np.zeros((128, 128), f32)
    for g in range(128 // GSIZE):
        # averages raw per-partition [sum, sumsq] into per-group [mean, E[x^2]]
        gmat[g * GSIZE : (g + 1) * GSIZE, g * GSIZE : (g + 1) * GSIZE] = 1.0 / (
            GSIZE * S
        )
    common["gmat"] = gmat

    in_maps = []
    for core in range(N_CORES):
        b_idx, half = divmod(core, 2)
        xb = hs[b_idx]
        if half:
            xp = np.concatenate([xb[:, SH:], xb[:, :SH]], axis=1)
        else:
            xp = xb
        m = dict(common)
        m["x"] = np.ascontiguousarray(xp.astype(np.float16))
        m["xr"] = np.ascontiguousarray(xp[:, :SH])
        in_maps.append(m)
    return in_maps


def assemble_output(results):
    out = np.empty((B, C, S), np.float32)
    for core in range(N_CORES):
        b_idx, half = divmod(core, 2)
        out[b_idx][:, half * SH : (half + 1) * SH] = results[core]["out"]
    return out.reshape(B, C, 64, 64)


def run(trace=False, **inputs):
    nc = _get_program()
    in_maps = make_in_maps(**inputs)
    res = run_bass_kernel_spmd(nc, in_maps, core_ids=list(range(N_CORES)), trace=trace)
    return assemble_output(res.results), res


def kernel(**inputs):
    out, _ = run(**inputs)
    return out


# revision 23
# speedup vs baseline: 1.3083x; 1.0122x over previous
"""Trainium2 Bass kernel for an AttentionBlock (GroupNorm -> 1-head attention -> proj -> residual).

Problem: hidden_states (4, 512, 64, 64) fp32; GroupNorm(32 groups) then
single-head attention over S=4096 tokens with head_dim=C=512, output
projection, residual add.

Sharding: 8 cores = 4 batch elements x 2 query-halves. Each core:
 - receives the full [512, 4096] (channels x spatial) slab for its batch
   element, spatially rotated so that *its* 2048 queries are columns 0:2048
   (attention is permutation-invariant over keys, so every core can run the
   identical SPMD program);
 - computes GroupNorm + K/V for all 4096 tokens (redundant x2, cheap) and
   Q only for its half;
 - computes scores^T (keys-on-partition layout), exp, attn @ V, out-proj,
   residual -- no on-chip transposes anywhere.

Numerics: fp8(e4m3) matmul operands with DoubleRow perf mode (two 128-row
k-tiles contracted per PE pass -> ~1.5-2x TensorE throughput) and fp32 PSUM
accumulation. Weights are pre-scaled on the host (wq,wk x16; wv x8; wo x16)
to keep fp8 operands out of the subnormal range; all scales cancel through
the softmax-denominator broadcast constant (ones = 8*16 = 128). Softmax
without max-subtraction (scores ~ N(0,1)) but with a constant exp-bias of -4
to keep unnormalized sums bounded; normalization deferred past the output
projection ((P@V)@Wo / den == (P/den @ V)@Wo).
"""

from contextlib import ExitStack

import ml_dtypes
import numpy as np

import concourse.bacc as bacc
import concourse.bass as bass
import concourse.tile as tile
from concourse import mybir
from concourse.bass_utils import run_bass_kernel_spmd

F32 = mybir.dt.float32
F16 = mybir.dt.float16
F8 = mybir.dt.float8e4
F8NP = ml_dtypes.float8_e4m3
DR = mybir.MatmulPerfMode.DoubleRow

B = 4
C = 512
S = 4096  # 64*64 tokens
SH = S // 2  # tokens per core (query half)
GROUPS = 32
GSIZE = C // GROUPS  # 16 channels per group
EPS = 1e-6
CT = C // 128  # 4 channel tiles
SCALE = 1.0 / np.sqrt(np.float32(C))
EXPBIAS = -4.0  # constant max-substitute inside exp; cancels in normalization

QKSCALE = 16.0  # host pre-scale on wq/wk/bq/bk (fp8 range use)
VSCALE = 4.0  # host pre-scale on wv/bv (keeps unnormalized attn@V in fp8 range)
OSCALE = 16.0  # host pre-scale on wo
ONES_VAL = VSCALE * OSCALE  # denominator broadcast constant; cancels v/o scales
EXPSCALE = float(SCALE / (QKSCALE * QKSCALE))  # exp() input scale on raw scores

N_CORES = 8


def _build_kernel(ctx: ExitStack, tc: tile.TileContext, d):
    nc = tc.nc
    mult = mybir.AluOpType.mult
    add = mybir.AluOpType.add
    subtract = mybir.AluOpType.subtract
    Act = mybir.ActivationFunctionType

    cst = ctx.enter_context(tc.tile_pool(name="cst", bufs=1))
    xin = ctx.enter_context(tc.tile_pool(name="xin", bufs=3))
    gnp = ctx.enter_context(tc.tile_pool(name="gnp", bufs=4))
    big = ctx.enter_context(tc.tile_pool(name="big", bufs=1))
    expp = ctx.enter_context(tc.tile_pool(name="expp", bufs=6))
    smal = ctx.enter_context(tc.tile_pool(name="smal", bufs=2))
    resp = ctx.enter_context(tc.tile_pool(name="resp", bufs=2))
    finp = ctx.enter_context(tc.tile_pool(name="finp", bufs=2))

    x_d = d["x"]  # fp16 copy of the input slab: GN stats + matmul path
    # sync DMA queue order: channel tile 0 first (it heads the GroupNorm
    # pipeline), then the tiny GN constants it needs, then the other tiles.
    # Four sub-DMAs per tile so bn_stats starts on the first quarter early;
    # each tile gets its own slot so all transfers issue immediately.
    x_tiles = []
    for t in range(CT):
        x_t = xin.tile([128, S], F16, tag=f"xt{t}", name=f"xt{t}", bufs=1)
        x_tiles.append(x_t)

    def dma_x(t):
        for h in range(4):
            nc.sync.dma_start(
                out=x_tiles[t][:, h * (S // 4) : (h + 1) * (S // 4)],
                in_=x_d[t * 128 : (t + 1) * 128, h * (S // 4) : (h + 1) * (S // 4)],
            )

    dma_x(0)
    gmat_raw = cst.tile([128, 128], F32, tag="gmat_raw")
    nc.sync.dma_start(out=gmat_raw[:], in_=d["gmat"][:])
    gw_sb = cst.tile([128, CT], F32, tag="gw")
    nc.sync.dma_start(out=gw_sb[:], in_=d["gw2"][:])
    gb_sb = cst.tile([128, CT], F32, tag="gb")
    nc.sync.dma_start(out=gb_sb[:], in_=d["gb2"][:])
    for t in range(1, CT):
        dma_x(t)

    # ---- constants / weights to SBUF (gpsimd DMA queue; overlaps x).
    # Order = first-use order: K/Q/V weights gate the projections,
    # biases gate the PSUM->SBUF copies a bit later, wo3/bo much later.
    wq3 = cst.tile([128, CT, C], F8, tag="wq3")
    wk3 = cst.tile([128, CT, C], F8, tag="wk3")
    wv3 = cst.tile([128, CT, C], F8, tag="wv3")
    wo3 = cst.tile([128, CT, C], F8, tag="wo3")
    for w_sb, w_d in ((wk3, d["wkt"]), (wq3, d["wqt"]), (wv3, d["wvt"])):
        nc.gpsimd.dma_start(out=w_sb[:], in_=w_d.rearrange("(t p) o -> p t o", p=128))
    bq_sb = cst.tile([128, CT], F32, tag="bq")
    bk_sb = cst.tile([128, CT], F32, tag="bk")
    bo_sb = cst.tile([128, CT], F32, tag="bo")
    for t_sb, t_d in ((bk_sb, d["bk2"]), (bq_sb, d["bq2"]), (bo_sb, d["bo2"])):
        nc.gpsimd.dma_start(out=t_sb[:], in_=t_d[:])
    nc.gpsimd.dma_start(out=wo3[:], in_=d["wot"].rearrange("(t p) o -> p t o", p=128))
    # staging copy: the first PE matmul then depends only on the DVE
    # semaphore (S3_LW allows a single wait)
    gmat_sb = cst.tile([128, 128], F32, tag="gmat")
    nc.vector.tensor_copy(out=gmat_sb[:], in_=gmat_raw[:])
    ones8 = cst.tile([128, 2, 128], F8, tag="ones8")
    nc.vector.memset(ones8[:], float(ONES_VAL))
    eps_t = cst.tile([128, 1], F32, tag="epsc")
    nc.vector.memset(eps_t[:], float(EPS))
    expb_t = cst.tile([128, 1], F32, tag="expb")
    nc.vector.memset(expb_t[:], float(EXPBIAS))

    # proj-phase PSUM pool: 6 banks; scoped so its banks are released to the
    # attention pools afterwards
    proj_ctx = ExitStack()
    pjsum = proj_ctx.enter_context(tc.tile_pool(name="pjsum", bufs=6, space="PSUM"))

    # PE warmup: keep TensorE busy during the initial x DMA so HAM reaches
    # K=8/8 before real matmuls; fp16 ones matmuls, one PSUM bank, serial.
    wu = pjsum.tile([128, 128], F32, tag="wu", bufs=1)
    for _ in range(160):
        nc.tensor.matmul(
            wu[:], lhsT=ones8[:, 0, :], rhs=ones8[:, 0, :], start=True, stop=True
        )
    # ---- GroupNorm ----
    # Pass 1: per-partition raw [sum, sumsq] for ALL tiles -- the plain sum on
    # DVE (reduce) and the sum of squares on ACT (Square activation with
    # accum_out; its junk output lands in the xg3 slot, which the normalize
    # pass overwrites), then the group-averaging matmul. Keeping the four
    # reduces back-to-back on DVE (no per-tile chain interleaved) shortens the
    # stats pipeline by several us. The 1/(group_size*S) normalization is
    # folded into the host-provided gmat constants.
    xg3 = big.tile([128, CT, S], F8, tag="xg3")  # normalized input, [c, s]
    ps_gs = []
    for t in range(CT):
        x_t = x_tiles[t]
        mv2 = gnp.tile([128, 2], F32, tag=f"mv2_{t}", name=f"mv2_{t}", bufs=1)
        # sum of squares per QUARTER so ACT starts as soon as each quarter's
        # DMA lands (the four Square passes pipeline with the x transfers
        # instead of serializing after them); junk main output lands in the
        # xg3 slot, which the normalize pass overwrites.
        sq4 = gnp.tile([128, 4], F32, tag=f"sq4_{t}", name=f"sq4_{t}", bufs=1)
        for h in range(4):
            nc.scalar.activation(
                out=xg3[:, t, h * 1024 : (h + 1) * 1024],
                in_=x_t[:, h * 1024 : (h + 1) * 1024],
                func=Act.Square,
                accum_out=sq4[:, h : h + 1],
            )
        # two fp16 pairwise-fold stages (DVE 2x eligible) before the 1x final
        # reduce: ~2.7us instead of 4.4us per tile on the DVE startup chain.
        # fp16 rounding in the folds perturbs the mean by ~1e-5 -- negligible.
        sc = gnp.tile([128, 2048], F16, tag="redsc", name="redsc", bufs=2)
        nc.vector.tensor_add(out=sc[:], in0=x_t[:, 0:2048], in1=x_t[:, 2048:4096])
        nc.vector.tensor_add(out=sc[:, 0:1024], in0=sc[:, 0:1024], in1=sc[:, 1024:2048])
        nc.vector.reduce_sum(out=mv2[:, 0:1], in_=sc[:, 0:1024], axis=mybir.AxisListType.X)
        nc.vector.reduce_sum(out=mv2[:, 1:2], in_=sq4[:], axis=mybir.AxisListType.X)
        ps_g = pjsum.tile([128, 2], F32, tag="pj", name=f"ps_g{t}")
        nc.tensor.matmul(ps_g[:], lhsT=gmat_sb[:], rhs=mv2[:], start=True, stop=True)
        ps_gs.append(ps_g)

    # Pass 2: per-tile scale/shift chain + normalize
    for t in range(CT):
        x_t = x_tiles[t]
        ps_g = ps_gs[t]
        # gstat = [mean_g, E[x^2]_g];  var_g = E[x^2]_g - mean_g^2
        gstat = gnp.tile([128, 2], F32, tag="gstat")
        nc.vector.tensor_copy(out=gstat[:], in_=ps_g[:])
        varg = gnp.tile([128, 1], F32, tag="varg")
        nc.vector.tensor_tensor(out=varg[:], in0=gstat[:, 0:1], in1=gstat[:, 0:1], op=mult)
        nc.vector.tensor_tensor(out=varg[:], in0=gstat[:, 1:2], in1=varg[:], op=subtract)
        stdt = gnp.tile([128, 1], F32, tag="stdt")
        nc.scalar.activation(out=stdt[:], in_=varg[:], func=Act.Sqrt, bias=eps_t[:])
        rstd = gnp.tile([128, 1], F32, tag="rstd")
        nc.vector.reciprocal(out=rstd[:], in_=stdt[:])

        scl = gnp.tile([128, 1], F32, tag="scl")
        nc.vector.tensor_tensor(out=scl[:], in0=rstd[:], in1=gw_sb[:, t : t + 1], op=mult)
        sft = gnp.tile([128, 1], F32, tag="sft")
        nc.vector.tensor_tensor(out=sft[:], in0=gstat[:, 0:1], in1=scl[:], op=mult)
        nc.vector.tensor_tensor(out=sft[:], in0=gb_sb[:, t : t + 1], in1=sft[:], op=subtract)

        # normalize split 1:3 ACT:DVE -- the DVE fp16 tensor_scalar runs ~3x
        # faster per element than ACT Identity here, and ACT is already loaded
        # with the Square stats passes
        nc.scalar.activation(
            out=xg3[:, t, 0 : S // 4],
            in_=x_t[:, 0 : S // 4],
            func=Act.Identity,
            bias=sft[:],
            scale=scl[:],
        )
        nc.vector.tensor_scalar(
            out=xg3[:, t, S // 4 : S],
            in0=x_t[:, S // 4 : S],
            scalar1=scl[:],
            scalar2=sft[:],
            op0=mult,
            op1=add,
        )

    # ---- projections (fp8 DoubleRow: contract channel-tile pairs) ----
    kt3 = big.tile([128, CT, S], F8, tag="kt3")  # k^T [c, j], x QKSCALE
    for ot in range(CT):
        for jc in range(S // 512):
            ps = pjsum.tile([128, 512], F32, tag="pj")
            for tp in range(CT // 2):
                nc.tensor.matmul(
                    ps[:],
                    lhsT=wk3[:, 2 * tp : 2 * tp + 2, ot * 128 : (ot + 1) * 128],
                    rhs=xg3[:, 2 * tp : 2 * tp + 2, jc * 512 : (jc + 1) * 512],
                    start=(tp == 0),
                    stop=(tp == CT // 2 - 1),
                    perf_mode=DR,
                )
            j0 = jc * 512
            nc.scalar.activation(
                out=kt3[:, ot, j0 : j0 + 192],
                in_=ps[:, 0:192],
                func=Act.Identity,
                bias=bk_sb[:, ot : ot + 1],
            )
            nc.vector.tensor_scalar(
                out=kt3[:, ot, j0 + 192 : j0 + 512],
                in0=ps[:, 192:512],
                scalar1=bk_sb[:, ot : ot + 1],
                scalar2=None,
                op0=add,
            )

    qt3 = big.tile([128, CT, SH], F8, tag="qt3")  # q^T [c, i], x QKSCALE
    for ot in range(CT):
        for ic in range(SH // 512):
            ps = pjsum.tile([128, 512], F32, tag="pj")
            for tp in range(CT // 2):
                nc.tensor.matmul(
                    ps[:],
                    lhsT=wq3[:, 2 * tp : 2 * tp + 2, ot * 128 : (ot + 1) * 128],
                    rhs=xg3[:, 2 * tp : 2 * tp + 2, ic * 512 : (ic + 1) * 512],
                    start=(tp == 0),
                    stop=(tp == CT // 2 - 1),
                    perf_mode=DR,
                )
            i0 = ic * 512
            nc.scalar.activation(
                out=qt3[:, ot, i0 : i0 + 192],
                in_=ps[:, 0:192],
                func=Act.Identity,
                bias=bq_sb[:, ot : ot + 1],
            )
            nc.vector.tensor_scalar(
                out=qt3[:, ot, i0 + 192 : i0 + 512],
                in0=ps[:, 192:512],
                scalar1=bq_sb[:, ot : ot + 1],
                scalar2=None,
                op0=add,
            )

    v3 = big.tile([128, S // 128, C], F8, tag="v3")  # v natural [j, o], x VSCALE
    for jb in range(S // 128):
        ps = pjsum.tile([128, 512], F32, tag="pj")
        for tp in range(CT // 2):
            nc.tensor.matmul(
                ps[:],
                lhsT=xg3[:, 2 * tp : 2 * tp + 2, jb * 128 : (jb + 1) * 128],
                rhs=wv3[:, 2 * tp : 2 * tp + 2, :],
                start=(tp == 0),
                stop=(tp == CT // 2 - 1),
                perf_mode=DR,
            )
        nc.vector.tensor_copy(out=v3[:, jb, 0:256], in_=ps[:, 0:256])
        nc.scalar.activation(out=v3[:, jb, 256:512], in_=ps[:, 256:512], func=Act.Copy)

    # release the 6 proj banks, then open the attention PSUM pools:
    # ps pairs (2 banks x 2 bufs) + av0..3 (1 each) = 8 banks. The finisher's
    # denominator/out-proj PSUM shares the "ps" rotation.
    proj_ctx.close()
    ppsum = ctx.enter_context(tc.tile_pool(name="ppsum", bufs=3, space="PSUM"))
    dpsum = ctx.enter_context(tc.tile_pool(name="dpsum", bufs=1, space="PSUM"))
    apsum = ctx.enter_context(tc.tile_pool(name="apsum", bufs=1, space="PSUM"))

    # ---- attention + output projection, per 512-query chunk ----
    # The per-chunk epilogue (denominator, attn-out copies, output projection,
    # residual) is deferred into the next chunk's j-loop so its PE work and
    # PSUM->SBUF copies overlap the next chunk's score matmuls.
    NJP = S // 256  # 16 key-block pairs

    def make_finisher(ic, av, den_ps):
        isl = slice(ic * 512, (ic + 1) * 512)
        state = {}

        def finish_a():
            # PSUM->SBUF attn-out copies gate the next chunk's attnV (av bank
            # reuse): split DVE/GPSIMD so the ACT exp stream is not delayed.
            a4 = smal.tile([128, CT, 512], F8, tag="a4", name="a4")
            for ot in range(CT):
                nc.vector.tensor_copy(out=a4[:, ot, :], in_=av[ot][:])
            # reciprocal straight off the PE-accumulated denominator bank
            recip = smal.tile([128, 512], F32, tag="recip", name="recip")
            nc.vector.reciprocal(out=recip[:], in_=den_ps[:])
            state["recip"] = recip
            state["a4"] = a4

        def finish_b():
            recip, a4 = state["recip"], state["a4"]
            for ot2 in range(CT):
                osl = slice(ot2 * 128, (ot2 + 1) * 128)
                ps_o = ppsum.tile([128, 512], F32, tag="ps", name="ps_o")
                for tp in range(CT // 2):
                    nc.tensor.matmul(
                        ps_o[:],
                        lhsT=wo3[:, 2 * tp : 2 * tp + 2, osl],
                        rhs=a4[:, 2 * tp : 2 * tp + 2, :],
                        start=(tp == 0),
                        stop=(tp == CT // 2 - 1),
                        perf_mode=DR,
                    )
                res_t = resp.tile([128, 512], F32, tag="res", name="res_t")
                nc.sync.dma_start(out=res_t[:], in_=d["xr"][osl, isl])
                f1 = finp.tile([128, 512], F32, tag="f1", name="f1")
                nc.vector.tensor_tensor(out=f1[:], in0=ps_o[:], in1=recip[:], op=mult)
                nc.vector.scalar_tensor_tensor(
                    out=f1[:],
                    in0=f1[:],
                    scalar=bo_sb[:, ot2 : ot2 + 1],
                    in1=res_t[:],
                    op0=add,
                    op1=add,
                )
                nc.sync.dma_start(out=d["out"][osl, isl], in_=f1[:])

        return finish_a, finish_b

    finish_prev = None
    for ic in range(SH // 512):
        isl = slice(ic * 512, (ic + 1) * 512)
        av = [
            apsum.tile([128, 512], F32, tag=f"av{ot}", name=f"av{ot}")
            for ot in range(CT)
        ]
        den_ps = dpsum.tile([128, 512], F32, tag="den", name="den_ps")

        def scores_exp(jp):
            # scores^T for key blocks 2jp, 2jp+1: two single-bank PSUM tiles,
            # two 512-wide exps into the fp8 pair-layout tile attnV reads.
            e_t = expp.tile([128, 2, 512], F8, tag="exp", name="e_t")
            for h in range(2):
                jb = 2 * jp + h
                ps_s = ppsum.tile([128, 512], F32, tag="ps", name="ps_s")
                for tp in range(CT // 2):
                    nc.tensor.matmul(
                        ps_s[:],
                        lhsT=kt3[:, 2 * tp : 2 * tp + 2, jb * 128 : (jb + 1) * 128],
                        rhs=qt3[:, 2 * tp : 2 * tp + 2, isl],
                        start=(tp == 0),
                        stop=(tp == CT // 2 - 1),
                        perf_mode=DR,
                    )
                nc.scalar.activation(
                    out=e_t[:, h, :],
                    in_=ps_s[:],
                    func=Act.Exp,
                    bias=expb_t[:],
                    scale=EXPSCALE,
                )
            return e_t

        es = {}

        def attnv_sums(jp, e_t):
            for ot in range(CT):
                nc.tensor.matmul(
                    av[ot][:],
                    lhsT=v3[:, 2 * jp : 2 * jp + 2, ot * 128 : (ot + 1) * 128],
                    rhs=e_t[:],
                    start=(jp == 0),
                    stop=(jp == NJP - 1),
                    perf_mode=DR,
                )
            # denominator accumulation on PE, one pair behind (gives the
            # previous chunk's reciprocal time to read the den bank)
            if jp > 0:
                nc.tensor.matmul(
                    den_ps[:],
                    lhsT=ones8[:],
                    rhs=es[jp - 1][:],
                    start=(jp == 1),
                    stop=False,
                    perf_mode=DR,
                )

        # Depth-3 score pipeline across the chunk boundary: three pair-groups
        # of scores queue on PE before the first attnV (which must wait for
        # the previous chunk's a4 copies to release the av banks).
        es[0] = scores_exp(0)
        es[1] = scores_exp(1)
        if finish_prev is not None:
            finish_prev[0]()
        es[2] = scores_exp(2)
        es[3] = scores_exp(3)
        for jp in range(NJP):
            attnv_sums(jp, es[jp])
            if jp + 4 < NJP:
                es[jp + 4] = scores_exp(jp + 4)
            if jp == 3 and finish_prev is not None:
                finish_prev[1]()
                finish_prev = None
        nc.tensor.matmul(
            den_ps[:],
            lhsT=ones8[:],
            rhs=es[NJP - 1][:],
            start=False,
            stop=True,
            perf_mode=DR,
        )
        es.clear()
        finish_prev = make_finisher(ic, av, den_ps)
    finish_prev[0]()
    finish_prev[1]()


_CACHE = {}


def _get_program():
    if "nc" in _CACHE:
        return _CACHE["nc"]
    nc = bacc.Bacc("TRN2", target_bir_lowering=False, debug=False, num_devices=N_CORES)
    d = {}
    d["x"] = nc.dram_tensor("x", [C, S], F16, kind="ExternalInput").ap()
    d["xr"] = nc.dram_tensor("xr", [C, SH], F32, kind="ExternalInput").ap()
    for name in ("wqt", "wkt", "wvt", "wot"):
        d[name] = nc.dram_tensor(name, [C, C], F8, kind="ExternalInput").ap()
    for name in ("bq2", "bk2", "bo2", "gw2", "gb2"):
        d[name] = nc.dram_tensor(name, [128, CT], F32, kind="ExternalInput").ap()
    d["gmat"] = nc.dram_tensor("gmat", [128, 128], F32, kind="ExternalInput").ap()
    d["out"] = nc.dram_tensor("out", [C, SH], F32, kind="ExternalOutput").ap()

    with tile.TileContext(nc) as tc:
        with ExitStack() as ctx:
            _build_kernel(ctx, tc, d)
    nc.compile()
    _CACHE["nc"] = nc
    return nc


def make_in_maps(**inputs):
    """Per-core input dicts (numpy). Core c handles batch c//2, query-half c%2."""
    f32 = np.float32
    hs = np.asarray(inputs["hidden_states"], f32).reshape(B, C, S)
    common = {}
    for wname, key, ws in (
        ("wqt", "wq", QKSCALE),
        ("wkt", "wk", QKSCALE),
        ("wvt", "wv", VSCALE),
        ("wot", "wo", OSCALE),
    ):
        w = np.asarray(inputs[key], f32) * ws
        common[wname] = np.ascontiguousarray(w.T).astype(F8NP)
    # bv folds into the output-projection bias: out += (P @ 1*bv) @ Wo.T
    bo_eff = np.asarray(inputs["bo"], f32) + np.asarray(inputs["wo"], f32) @ np.asarray(
        inputs["bv"], f32
    )
    for bname, bvec, bs in (
        ("bq2", np.asarray(inputs["bq"], f32), QKSCALE),
        ("bk2", np.asarray(inputs["bk"], f32), QKSCALE),
        ("bo2", bo_eff, 1.0),
    ):
        b = bvec * bs
        common[bname] = np.ascontiguousarray(b.reshape(CT, 128).T)
    common["gw2"] = np.ascontiguousarray(
        np.asarray(inputs["gn_weight"], f32).reshape(CT, 128).T
    )
    common["gb2"] = np.ascontiguousarray(
        np.asarray(inputs["gn_bias"], f32).reshape(CT, 128).T
    )
    gmat = np.zeros((128, 128), f32)
    for g in range(128 // GSIZE):
        # averages raw per-partition [sum, sumsq] into per-group [mean, E[x^2]]
        gmat[g * GSIZE : (g + 1) * GSIZE, g * GSIZE : (g + 1) * GSIZE] = 1.0 / (
            GSIZE * S
        )
    common["gmat"] = gmat

    in_maps = []
    for core in range(N_CORES):
        b_idx, half = divmod(core, 2)
        xb = hs[b_idx]
        if half:
            xp = np.concatenate([xb[:, SH:], xb[:, :SH]], axis=1)
        else:
            xp = xb
        m = dict(common)
        m["x"] = np.ascontiguousarray(xp.astype(np.float16))
        m["xr"] = np.ascontiguousarray(xp[:, :SH])
        in_maps.append(m)
    return in_maps


def assemble_output(results):
    out = np.empty((B, C, S), np.float32)
    for core in range(N_CORES):
        b_idx, half = divmod(core, 2)
        out[b_idx][:, half * SH : (half + 1) * SH] = results[core]["out"]
    return out.reshape(B, C, 64, 64)


def run(trace=False, **inputs):
    nc = _get_program()
    in_maps = make_in_maps(**inputs)
    res = run_bass_kernel_spmd(nc, in_maps, core_ids=list(range(N_CORES)), trace=trace)
    return assemble_output(res.results), res


def kernel(**inputs):
    out, _ = run(**inputs)
    return out
